# revision 4
# baseline (speedup 1.0000x reference)
"""Jamba sparse-MoE block on 8 Trainium2 NeuronCores (expert parallelism).

Contract: kernel(**inputs) takes the FULL unsharded inputs
(hidden_states [2,1024,1024] f32, router_w [8,1024], gate_w/up_w [8,2816,1024],
down_w [8,1024,2816]) and returns the FULL output [2,1024,1024] f32.

Strategy (v1, dense): one expert per core; tokens replicated. Each core:
  - computes router logits in f32 (top-2 selection must match the f32
    reference bit-for-bit in selection: min p2-p3 gap is ~8.5e-5),
  - computes its expert's gated MLP densely over all 2048 tokens in bf16
    (feature-major layout [feat, tok] so no activation transposes are needed),
  - scales the output columns by the per-token routing weight (zero for
    tokens that did not pick this expert),
  - ReduceScatter(add) over the 8 cores combines the expert contributions;
    each core emits a [128, 2048] H-slice of y^T, gathered on host.
"""

import numpy as np
import ml_dtypes

import concourse.bass as bass
import concourse.mybir as mybir
import concourse.tile as tile
from concourse import bacc
from concourse.bass_utils import run_bass_kernel_spmd
from concourse.masks import make_identity

H = 1024       # hidden size
I = 2816       # intermediate size
E = 8          # num experts
T = 2048       # tokens (2*1024)
P = 128
N_CORES = 8
KT = H // P    # 8  k-tiles over H
MI = I // P    # 22 m-tiles over I
TCH = 1024     # token chunk for gate/up/down
NTC = T // TCH

F32 = mybir.dt.float32
BF16 = mybir.dt.bfloat16
AX = mybir.AxisListType.X
ALU = mybir.AluOpType
ACT_FN = mybir.ActivationFunctionType


def _bf16(a: np.ndarray) -> np.ndarray:
    """Fast float32 -> bfloat16 (round to nearest even)."""
    a = np.ascontiguousarray(a, dtype=np.float32)
    v = a.view(np.uint32)
    r = ((v >> 16) & 1) + np.uint32(0x7FFF)
    out = ((v + r) >> 16).astype(np.uint16)
    return out.view(ml_dtypes.bfloat16)


def build_nc():
    nc = bacc.Bacc("TRN2", target_bir_lowering=False, debug=False,
                   num_devices=N_CORES)
    xt = nc.dram_tensor("xt", [H, T], F32, kind="ExternalInput")
    rwt = nc.dram_tensor("rwt", [H, E], F32, kind="ExternalInput")
    sel = nc.dram_tensor("sel", [P, E], F32, kind="ExternalInput")
    gwt = nc.dram_tensor("gwt", [H, I], BF16, kind="ExternalInput")
    uwt = nc.dram_tensor("uwt", [H, I], BF16, kind="ExternalInput")
    dwt = nc.dram_tensor("dwt", [I, H], BF16, kind="ExternalInput")
    out = nc.dram_tensor("out", [P, T], F32, kind="ExternalOutput")

    with tile.TileContext(nc) as tc:
        _build(tc, xt, rwt, sel, gwt, uwt, dwt, out)
    nc.compile()
    return nc


def _build(tc, xt, rwt, sel, gwt, uwt, dwt, out_ext):
    nc = tc.nc
    with (
        tc.tile_pool(name="const", bufs=1) as const,
        tc.tile_pool(name="x32p", bufs=2) as x32p,
        tc.tile_pool(name="xbfp", bufs=KT) as xbfp,
        tc.tile_pool(name="lsbp", bufs=1) as lsbp,
        tc.tile_pool(name="wsbp", bufs=1) as wsbp,
        tc.tile_pool(name="smaxp", bufs=2) as smaxp,
        tc.tile_pool(name="hallp", bufs=1) as hallp,
        tc.tile_pool(name="gactp", bufs=2) as gactp,
        tc.tile_pool(name="wtp", bufs=8) as wtp,
        tc.tile_pool(name="ysbp", bufs=2) as ysbp,
        tc.tile_pool(name="dramp", bufs=1, space="DRAM") as dramp,
    ):
        # ---- constants
        ident = const.tile([P, P], F32, name="ident")
        make_identity(nc, ident)
        ones1 = const.tile([1, P], F32, name="ones1")
        nc.vector.memset(ones1[:], 1.0)
        sel_sb = const.tile([P, E], F32, name="sel_sb")
        nc.sync.dma_start(out=sel_sb[:], in_=sel[:])
        rwt_sb = const.tile([P, KT * E], F32, name="rwt_sb")
        for k in range(KT):
            nc.sync.dma_start(out=rwt_sb[:, k * E:(k + 1) * E],
                              in_=rwt[k * P:(k + 1) * P, :])

        partial = dramp.tile([H, T], F32, name="partial")
        rs_out = dramp.tile([P, T], F32, name="rs_out")

        # ---- load x^T, cast to bf16, router logits (transposed: [E, T])
        xbf = []
        with tc.tile_pool(name="psum_lt", bufs=4, space="PSUM") as pltp:
            lt_ps = [pltp.tile([E, 512], F32, name=f"lt{n}", tag="lt")
                     for n in range(4)]
            for k in range(KT):
                x32 = x32p.tile([P, T], F32, tag="x32")
                nc.sync.dma_start(out=x32[:], in_=xt[k * P:(k + 1) * P, :])
                xb = xbfp.tile([P, T], BF16, tag="xb")
                nc.vector.tensor_copy(out=xb[:], in_=x32[:])
                xbf.append(xb)
                for n in range(4):
                    nc.tensor.matmul(
                        lt_ps[n][:],
                        rwt_sb[:, k * E:(k + 1) * E],
                        x32[:, n * 512:(n + 1) * 512],
                        start=(k == 0), stop=(k == KT - 1),
                    )
            lt_sb = lsbp.tile([E, T], F32, name="lt_sb")
            for n in range(4):
                nc.scalar.copy(out=lt_sb[:, n * 512:(n + 1) * 512],
                               in_=lt_ps[n][:])

        # ---- transpose logits to [tok-part, 16*E]
        with tc.tile_pool(name="psum_r", bufs=1, space="PSUM") as prp:
            l_ps = prp.tile([P, P], F32, name="l_ps")
            for m in range(16):
                nc.tensor.transpose(l_ps[:, m * E:(m + 1) * E],
                                    lt_sb[:, m * P:(m + 1) * P],
                                    ident[:E, :E])
            l_sb = lsbp.tile([P, P], F32, name="l_sb")
            nc.scalar.copy(out=l_sb[:], in_=l_ps[:])

        # ---- softmax + top-2 weight for this core's expert
        # logits are small (|l| < ~5): exp() without max-subtraction is safe.
        w_all = wsbp.tile([P, 16], F32, name="w_all")
        for m in range(16):
            l = l_sb[:, m * E:(m + 1) * E]
            el = smaxp.tile([P, E], F32, tag="el")
            nc.scalar.activation(out=el[:], in_=l, func=ACT_FN.Exp)
            s = smaxp.tile([P, 1], F32, tag="s")
            nc.vector.reduce_sum(s[:], el[:], axis=AX)
            m1 = smaxp.tile([P, 1], F32, tag="m1")
            nc.vector.reduce_max(m1[:], el[:], axis=AX)
            mask1 = smaxp.tile([P, E], F32, tag="mask1")
            nc.vector.tensor_scalar(out=mask1[:], in0=el[:], scalar1=m1[:],
                                    scalar2=None, op0=ALU.is_lt)
            pwo = smaxp.tile([P, E], F32, tag="pwo")
            nc.vector.tensor_mul(pwo[:], el[:], mask1[:])
            m2 = smaxp.tile([P, 1], F32, tag="m2")
            nc.vector.reduce_max(m2[:], pwo[:], axis=AX)
            sel2 = smaxp.tile([P, E], F32, tag="sel2")
            nc.vector.tensor_scalar(out=sel2[:], in0=el[:], scalar1=m2[:],
                                    scalar2=None, op0=ALU.is_ge)
            wte = smaxp.tile([P, E], F32, tag="wte")
            nc.vector.tensor_mul(wte[:], el[:], sel2[:])
            nc.vector.tensor_mul(wte[:], wte[:], sel_sb[:])
            wsum = smaxp.tile([P, 1], F32, tag="wsum")
            nc.vector.reduce_sum(wsum[:], wte[:], axis=AX)
            rs = smaxp.tile([P, 1], F32, tag="rs")
            nc.vector.reciprocal(rs[:], s[:])
            nc.vector.tensor_mul(w_all[:, m:m + 1], wsum[:], rs[:])

        h_all = hallp.tile([P, MI * TCH], BF16, name="h_all")
        w_bc = wsbp.tile([P, T], F32, name="w_bc")

        for t_c in range(NTC):
            t0 = t_c * TCH
            # ---- gate & up projections: G_t/U_t [I, TCH] bf16 matmuls
            with (
                tc.tile_pool(name=f"psg{t_c}", bufs=2, space="PSUM") as pgp,
                tc.tile_pool(name=f"psu{t_c}", bufs=2, space="PSUM") as pup,
            ):
                for m in range(MI):
                    pg = pgp.tile([P, TCH], F32, tag="pg")
                    pu = pup.tile([P, TCH], F32, tag="pu")
                    for k in range(KT):
                        wg = wtp.tile([P, P], BF16, tag="wg")
                        nc.sync.dma_start(
                            out=wg[:], in_=gwt[k * P:(k + 1) * P, m * P:(m + 1) * P])
                        for n in range(TCH // 512):
                            nc.tensor.matmul(
                                pg[:, n * 512:(n + 1) * 512], wg[:],
                                xbf[k][:, t0 + n * 512:t0 + (n + 1) * 512],
                                start=(k == 0), stop=(k == KT - 1))
                    for k in range(KT):
                        wu = wtp.tile([P, P], BF16, tag="wu")
                        nc.sync.dma_start(
                            out=wu[:], in_=uwt[k * P:(k + 1) * P, m * P:(m + 1) * P])
                        for n in range(TCH // 512):
                            nc.tensor.matmul(
                                pu[:, n * 512:(n + 1) * 512], wu[:],
                                xbf[k][:, t0 + n * 512:t0 + (n + 1) * 512],
                                start=(k == 0), stop=(k == KT - 1))
                    ga = gactp.tile([P, TCH], F32, tag="ga")
                    nc.scalar.activation(out=ga[:], in_=pg[:], func=ACT_FN.Silu)
                    nc.vector.tensor_mul(
                        h_all[:, m * TCH:(m + 1) * TCH], ga[:], pu[:])

            # ---- once: broadcast routing weights to [P, T]
            if t_c == 0:
                with tc.tile_pool(name="psum_b", bufs=1, space="PSUM") as pbp:
                    wt_ps = pbp.tile([16, P], F32, name="wt_ps")
                    nc.tensor.transpose(wt_ps[:], w_all[:], ident[:])
                    wt_sb = wsbp.tile([16, P], F32, name="wt_sb")
                    nc.scalar.copy(out=wt_sb[:], in_=wt_ps[:])
                    w_row = wsbp.tile([1, T], F32, name="w_row")
                    for i in range(16):
                        nc.sync.dma_start(out=w_row[0:1, i * P:(i + 1) * P],
                                          in_=wt_sb[i:i + 1, :])
                    for n in range(T // 512):
                        pb = pbp.tile([P, 512], F32, tag="pb")
                        nc.tensor.matmul(pb[:], ones1[:],
                                         w_row[0:1, n * 512:(n + 1) * 512],
                                         start=True, stop=True)
                        nc.scalar.copy(out=w_bc[:, n * 512:(n + 1) * 512],
                                       in_=pb[:])

            # ---- down projection + routing-weight scale -> partial [H, T]
            with tc.tile_pool(name=f"psy{t_c}", bufs=3, space="PSUM") as pyp:
                for mh in range(KT):
                    py = pyp.tile([P, TCH], F32, tag="py")
                    for k in range(MI):
                        wd = wtp.tile([P, P], BF16, tag="wd")
                        nc.sync.dma_start(
                            out=wd[:], in_=dwt[k * P:(k + 1) * P, mh * P:(mh + 1) * P])
                        for n in range(TCH // 512):
                            nc.tensor.matmul(
                                py[:, n * 512:(n + 1) * 512], wd[:],
                                h_all[:, k * TCH + n * 512:k * TCH + (n + 1) * 512],
                                start=(k == 0), stop=(k == MI - 1))
                    ysb = ysbp.tile([P, TCH], F32, tag="ysb")
                    nc.vector.tensor_mul(ysb[:], py[:], w_bc[:, t0:t0 + TCH])
                    nc.sync.dma_start(
                        out=partial[mh * P:(mh + 1) * P, t0:t0 + TCH], in_=ysb[:])

        # ---- combine expert contributions across cores
        nc.gpsimd.collective_compute(
            "ReduceScatter",
            ALU.add,
            replica_groups=[list(range(N_CORES))],
            ins=[partial.opt()],
            outs=[rs_out.opt()],
        )
        nc.sync.dma_start(out=out_ext[:], in_=rs_out[:])


_NC_CACHE = None


def _get_nc():
    global _NC_CACHE
    if _NC_CACHE is None:
        _NC_CACHE = build_nc()
    return _NC_CACHE


def make_in_maps(hidden_states, router_w, gate_w, up_w, down_w):
    x = np.ascontiguousarray(np.asarray(hidden_states, dtype=np.float32)
                             .reshape(T, H))
    xt = np.ascontiguousarray(x.T)                       # [H, T] f32
    rwt = np.ascontiguousarray(np.asarray(router_w, np.float32).T)  # [H, E]
    in_maps = []
    for e in range(N_CORES):
        sel = np.zeros((P, E), np.float32)
        sel[:, e] = 1.0
        in_maps.append({
            "xt": xt,
            "rwt": rwt,
            "sel": sel,
            "gwt": _bf16(np.asarray(gate_w[e], np.float32).T),   # [H, I]
            "uwt": _bf16(np.asarray(up_w[e], np.float32).T),     # [H, I]
            "dwt": _bf16(np.asarray(down_w[e], np.float32).T),   # [I, H]
        })
    return in_maps


def assemble_output(results):
    yt = np.concatenate([np.asarray(results[i]["out"], np.float32)
                         for i in range(N_CORES)], axis=0)   # [H, T]
    return np.ascontiguousarray(yt.T).reshape(2, T // 2, H)


def kernel(hidden_states, router_w, gate_w, up_w, down_w):
    nc = _get_nc()
    in_maps = make_in_maps(hidden_states, router_w, gate_w, up_w, down_w)
    res = run_bass_kernel_spmd(nc, in_maps, list(range(N_CORES)))
    return assemble_output(res.results)


def run_traced(inputs, **kw):
    """Used by test.py: run with NTFF profiling enabled.

    Warm up untraced first so neuronxcc compile + NEFF load happen outside
    the NRT profile session (profiling a cold load wedges the exec unit)."""
    nc = _get_nc()
    in_maps = make_in_maps(**inputs)
    run_bass_kernel_spmd(nc, in_maps, list(range(N_CORES)))
    res = run_bass_kernel_spmd(nc, in_maps, list(range(N_CORES)),
                               trace=True, **kw)
    return assemble_output(res.results), res
